# revision 12
# baseline (speedup 1.0000x reference)
"""Multi-head attention (B=4, N=2048, C=256, H=8) on 8 Trainium2 NeuronCores.

Sharding: core c handles batch b = c//2 and query-half qh = c%2 (1024 query
rows), all 8 heads. k/v are computed for the full sequence on each core (the
qkv projection is cheap); outputs concatenate with no cross-core reduction.

Device-side layout is fully "transposed" (channels on partitions):
  - x^T [C, N] feeds q^T/k^T ([d, tokens], head-major rows) and v ([tokens, d]).
  - Scores are computed as S^T [k-tokens, q-tokens] so that softmax's exp
    output E^T feeds the AV matmul directly (contraction over k on partitions).
  - Softmax denominators come for free as a 33rd "ones" column appended to v.
  - O^T [channels, q] feeds the output projection directly.
Softmax skips max-subtraction: scaled scores are ~N(0,1) (max |s| < ~10), safe
in fp32 exp. Matmuls run in float32r (full PE rate at free dim >= 256); every
matmul operand is produced by a rounding-capable instruction (DVE copy, ACT,
memset) with float32r output dtype, as the BIR verifier requires.
"""

import os
from contextlib import ExitStack

import numpy as np

import concourse.bacc as bacc
import concourse.bass as bass
import concourse.mybir as mybir
import concourse.tile as tile
from concourse.bass_utils import run_bass_kernel_spmd

B, N, C = 4, 2048, 256
H, D = 8, 32
P = 128
QH = N // 2              # query rows per core
SCALE = float(D) ** -0.5
NCORES = 8

F32 = mybir.dt.float32
F32R = mybir.dt.float32r
EXP = mybir.ActivationFunctionType.Exp

# Timing amplification: repeat the attention+proj body REPS times (same data,
# same output). Used only by the local test harness to measure per-rep HW time.
REPS = int(os.environ.get("BASS_ATTN_REPS", "1"))


def _emit(tc, xT, xTq, wqT, wkT, wvT, pwT, pb, y):
    nc = tc.nc
    with ExitStack() as ctx:
        singles = ctx.enter_context(tc.tile_pool(name="singles", bufs=1))
        epool = ctx.enter_context(tc.tile_pool(name="epool", bufs=3))
        small = ctx.enter_context(tc.tile_pool(name="small", bufs=4))
        ypool = ctx.enter_context(tc.tile_pool(name="ypool", bufs=3))
        ps = ctx.enter_context(tc.tile_pool(name="ps", bufs=3, space="PSUM"))
        po = ctx.enter_context(tc.tile_pool(name="po", bufs=2, space="PSUM"))

        # ---- input loads + fp32r rounding -------------------------------
        def load_r(name, dram_ap, cshape):
            ld = singles.tile(cshape, F32, tag=name + "_ld", name=name + "_ld")
            nc.sync.dma_start(ld[:], dram_ap)
            rt = singles.tile(cshape, F32R, tag=name, name=name)
            nc.vector.tensor_copy(rt[:], ld[:])
            return rt

        xT_sb = load_r("xT", xT.rearrange("(c p) n -> p c n", p=P), [P, 2, N])
        xTq_sb = load_r("xTq", xTq.rearrange("(c p) n -> p c n", p=P), [P, 2, QH])
        wq_sb = load_r("wq", wqT.rearrange("(c p) n -> p c n", p=P), [P, 2, C])
        wk_sb = load_r("wk", wkT.rearrange("(c p) n -> p c n", p=P), [P, 2, C])
        wv_sb = load_r("wv", wvT.rearrange("(c p) n -> p c n", p=P), [P, 2, C])
        pw_sb = load_r("pw", pwT.rearrange("(c p) n -> p c n", p=P), [P, 2, C])
        pb_sb = singles.tile([P, C], F32, tag="pb")
        nc.sync.dma_start(
            pb_sb[:],
            bass.AP(tensor=pb.tensor, offset=pb.offset, ap=[[0, P]] + list(pb.ap)),
        )

        # ---- qkv projections -------------------------------------------
        # q^T/k^T stacks: chunk cc holds heads 4cc..4cc+3 at rows 32*(h%4).
        qT_sb = singles.tile([P, 2, QH], F32R, tag="qT")
        kT_sb = singles.tile([P, 2, N], F32R, tag="kT")
        # v_aug: [token-tile, head-major (v_h | 1)] for AV + denominator.
        vA_sb = singles.tile([P, N // P, H * (D + 1)], F32R, tag="vA")
        # memset cannot produce fp32r; round 1.0s through a DVE copy instead
        onesF = singles.tile([P, N // P, H], F32, tag="onesF")
        nc.vector.memset(onesF[:], 1.0)
        vA4 = vA_sb[:].rearrange("p t (h a) -> p t h a", a=D + 1)
        nc.vector.tensor_copy(vA4[:, :, :, D], onesF[:])

        for cc in range(2):  # head-group chunk
            for nb in range(N // 512):  # k^T over full sequence
                pk = ps.tile([P, 1024], F32, tag="s", name="pk")
                for ci in range(2):
                    nc.tensor.matmul(
                        pk[:, 0:512],
                        lhsT=wk_sb[:, ci, 128 * cc : 128 * cc + 128],
                        rhs=xT_sb[:, ci, 512 * nb : 512 * nb + 512],
                        start=(ci == 0),
                        stop=(ci == 1),
                    )
                nc.vector.tensor_copy(
                    kT_sb[:, cc, 512 * nb : 512 * nb + 512], pk[:, 0:512]
                )
            for nb in range(QH // 512):  # q^T over this core's query half
                pq = ps.tile([P, 1024], F32, tag="s", name="pq")
                for ci in range(2):
                    nc.tensor.matmul(
                        pq[:, 0:512],
                        lhsT=wq_sb[:, ci, 128 * cc : 128 * cc + 128],
                        rhs=xTq_sb[:, ci, 512 * nb : 512 * nb + 512],
                        start=(ci == 0),
                        stop=(ci == 1),
                    )
                nc.vector.tensor_copy(
                    qT_sb[:, cc, 512 * nb : 512 * nb + 512], pq[:, 0:512]
                )
        for tt in range(N // P):  # v in [token, channel] layout
            pv = ps.tile([P, 1024], F32, tag="s", name="pv")
            for ci in range(2):
                nc.tensor.matmul(
                    pv[:, 0:256],
                    lhsT=xT_sb[:, ci, 128 * tt : 128 * tt + 128],
                    rhs=wv_sb[:, ci, :],
                    start=(ci == 0),
                    stop=(ci == 1),
                )
            nc.vector.tensor_copy(
                vA_sb[:, tt, :].rearrange("p (h a) -> p h a", a=D + 1)[:, :, 0:D],
                pv[:, 0:256].rearrange("p (h d) -> p h d", d=D),
            )

        # ---- attention + projection ------------------------------------
        ones_f2 = singles.tile([1, 32], F32, tag="onesf2")
        nc.vector.memset(ones_f2[:], 1.0)
        ones_sb = singles.tile([1, 32], F32R, tag="ones")
        nc.vector.tensor_copy(ones_sb[:], ones_f2[:])
        OT_sb = singles.tile([P, 2, QH], F32R, tag="OT")
        for _rep in range(REPS):
            for qb in range(QH // 512):
                for hp in range(H // 2):  # head pairs (2hp, 2hp+1)
                    pos = [
                        po.tile([D + 1, 512], F32, tag="o", name=f"po{e}")
                        for e in range(2)
                    ]
                    for ch in range(N // P):  # k chunks
                        s = ps.tile([P, 1024], F32, tag="s", name="s")
                        for e in range(2):
                            h = 2 * hp + e
                            r, cc = 32 * (h % 4), h // 4
                            nc.tensor.matmul(
                                s[:, 512 * e : 512 * e + 512],
                                lhsT=kT_sb[r : r + 32, cc, 128 * ch : 128 * ch + 128],
                                rhs=qT_sb[r : r + 32, cc, 512 * qb : 512 * qb + 512],
                                start=True,
                                stop=True,
                                tile_position=(r, 0),
                            )
                        et = epool.tile([P, 1024], F32R, tag="E", name="et")
                        nc.scalar.activation(et[:], s[:], EXP, scale=SCALE)
                        for e in range(2):
                            h = 2 * hp + e
                            nc.tensor.matmul(
                                pos[e][:],
                                lhsT=vA_sb[:, ch, (D + 1) * h : (D + 1) * (h + 1)],
                                rhs=et[:, 512 * e : 512 * e + 512],
                                start=(ch == 0),
                                stop=(ch == N // P - 1),
                            )
                    for e in range(2):
                        h = 2 * hp + e
                        r, cc = 32 * (h % 4), h // 4
                        rcpf = small.tile([1, 512], F32, tag="rcpf", name="rcpf")
                        nc.vector.reciprocal(rcpf[:], pos[e][D : D + 1, :])
                        rcp = small.tile([1, 512], F32R, tag="rcp", name="rcp")
                        nc.vector.tensor_copy(rcp[:], rcpf[:])
                        bc = ps.tile([P, 1024], F32, tag="s", name="bc")
                        nc.tensor.matmul(
                            bc[0:32, 0:512],
                            lhsT=ones_sb[:],
                            rhs=rcp[:],
                            start=True,
                            stop=True,
                        )
                        onr = small.tile([32, 512], F32, tag="onr", name="onr")
                        nc.vector.tensor_copy(onr[:], pos[e][0:D, :])
                        nc.vector.tensor_mul(
                            OT_sb[r : r + 32, cc, 512 * qb : 512 * qb + 512],
                            onr[:],
                            bc[0:32, 0:512],
                        )
                for qt in range(4):  # projection for this q-block
                    tq = 4 * qb + qt
                    py = ps.tile([P, 1024], F32, tag="s", name="py")
                    for ci in range(2):
                        nc.tensor.matmul(
                            py[:, 0:256],
                            lhsT=OT_sb[:, ci, 128 * tq : 128 * tq + 128],
                            rhs=pw_sb[:, ci, :],
                            start=(ci == 0),
                            stop=(ci == 1),
                        )
                    ysb = ypool.tile([P, C], F32, tag="y", name="ysb")
                    nc.vector.tensor_add(ysb[:], py[:, 0:256], pb_sb[:])
                    nc.sync.dma_start(y[128 * tq : 128 * tq + 128, :], ysb[:])


_NC = None


def _get_nc():
    global _NC
    if _NC is None:
        nc = bacc.Bacc("TRN2", target_bir_lowering=False, debug=False, num_devices=1)
        xT = nc.dram_tensor("xT", [C, N], F32, kind="ExternalInput").ap()
        xTq = nc.dram_tensor("xTq", [C, QH], F32, kind="ExternalInput").ap()
        wqT = nc.dram_tensor("wqT", [C, C], F32, kind="ExternalInput").ap()
        wkT = nc.dram_tensor("wkT", [C, C], F32, kind="ExternalInput").ap()
        wvT = nc.dram_tensor("wvT", [C, C], F32, kind="ExternalInput").ap()
        pwT = nc.dram_tensor("pwT", [C, C], F32, kind="ExternalInput").ap()
        pb = nc.dram_tensor("pb", [C], F32, kind="ExternalInput").ap()
        y = nc.dram_tensor("y", [QH, C], F32, kind="ExternalOutput").ap()
        with tile.TileContext(nc) as tc:
            _emit(tc, xT, xTq, wqT, wkT, wvT, pwT, pb, y)
        nc.finalize()
        _NC = nc
    return _NC


def kernel(x, qkv_w, proj_w, proj_b):
    x = np.asarray(x, dtype=np.float32)
    qkv_w = np.asarray(qkv_w, dtype=np.float32)
    proj_w = np.asarray(proj_w, dtype=np.float32)
    proj_b = np.asarray(proj_b, dtype=np.float32)

    nc = _get_nc()
    wqT = np.ascontiguousarray(qkv_w[0:C].T)
    wkT = np.ascontiguousarray(qkv_w[C : 2 * C].T)
    wvT = np.ascontiguousarray(qkv_w[2 * C : 3 * C].T)
    pwT = np.ascontiguousarray(proj_w.T)

    in_maps = []
    for c in range(NCORES):
        b, qh = c // 2, c % 2
        xT = np.ascontiguousarray(x[b].T)
        in_maps.append(
            {
                "xT": xT,
                "xTq": np.ascontiguousarray(xT[:, qh * QH : (qh + 1) * QH]),
                "wqT": wqT,
                "wkT": wkT,
                "wvT": wvT,
                "pwT": pwT,
                "pb": proj_b,
            }
        )
    res = run_bass_kernel_spmd(nc, in_maps, core_ids=list(range(NCORES)))
    out = np.empty((B, N, C), np.float32)
    for c in range(NCORES):
        b, qh = c // 2, c % 2
        out[b, qh * QH : (qh + 1) * QH] = res.results[c]["y"]
    return out
